# revision 1
# baseline (speedup 1.0000x reference)
"""Trainium2 Bass kernel for nn_AttnLoss_26551487823965.

Computes the attention MSE loss:
  loss = sum_c sgn_c * (cross_mse_c + sum_j gate[c,j] * Lj[j])
where Lj = mean over hw pixels of (self_attn - self_attn_erase)^2 per
pixel-column, gate = thresholded bilinear-resized attention masks, and
cross_mse = per-token-column MSE of attn vs attn_erase.

Sharding: self_attn/self_attn_erase are split along the last (hw=4096)
axis into 512 columns per core (8 cores).  Each core computes its
slice's column sums of (x-y)^2, the full mask gates (cheap, needs only
attn[:,:,1:9]), and the full cross term; the host sums the per-core
partial scalars.

Measured DMA facts on this hardware (dma_bench.py, async repeat-delta
through the axon relay, all 8 cores streaming):
 - per-core HBM->SBUF streaming rate ~374-389 GB/s,
 - each dma_start costs ~0.65 us of serial ring time regardless of size,
 - descriptor size (2 KB vs 16+ KB) costs only ~1-2 us per 16 MB.
So the design minimises DMA COUNT: x and y are concatenated by the host
into ONE column-major dram tensor (xy [1024, 4096]) so each streamed
chunk is a single dma_start (5 bulk DMAs, one per CHUNKS entry), and ALL
small inputs (identity/U^T/ones/sgn consts, U_max/U_need/E/dup matrices,
a8/e8/a8T packed pairwise in the partition dim) fit in ONE [128, 1545]
f32 tensor loaded by a single SWDGE DMA.  Merging two j-groups per DMA
(3 bulk DMAs of up to 8 MB) was tried and measured WORSE: the bigger
chunks delay first-compute and spill the second chunk's ACT work past
the final chunk's arrival, growing the drain more than the ~1.3 us of
saved per-DMA overhead.

Structure per core (all inside one Tile program).  The trn2 PE idles at
the 1.2 GHz pstate in a DMA-bound kernel, so the design keeps the bulk
path off PE entirely:
 - streaming loop on a COLUMN-major (host-pretransposed) layout:
   partition p = pixel column j = g*128+p (4 j-groups), free dim = hw
   rows.  One dma_start per chunk loads [x rows | y rows] as 2 contiguous
   descriptors per partition.  Subtract in place (DVE/Pool alternating),
   then ACT Square with accum_out reduces along the free dim -- the
   per-column sums sg [128,4] need NO PE matmuls.  The last chunk is
   tiny (128 rows) and its partial bypasses sg: the gate-dot base over
   everything else is precomputed, so the drain is sub -> ACT
   square-accum -> two in-order DVE ops -> out DMA.
 - cross-attn MSE + per-channel mask pipeline emitted FIRST so it
   overlaps the stream.  Mask math is reduced to its minimum: the global
   max of the 256-row upsample comes from the 128 extreme-blend rows
   (U_max, one matmul; interior blend weights are convex combinations of
   the a=.125/.875 rows); threshold bits are needed only at this core's
   16 rows x 128 cols (U_need, one tiny matmul); E pair-sums adjacent
   rows, a strided DVE add pair-sums adjacent cols, and gate = s2 > 0.
 - w8 [8,64] is flipped to wfl [128,4] (the sg layout) via PE transpose +
   a dup-matrix matmul + two strided DVE copies.
 - tail: sw = sg (*) wfl, reduce -> per-partition partials written
   straight into the [128,2] output and DMA'd out; the HOST sums the
   128 partials per core (it already sums the 8 cores), removing the
   PE partition-sum + ACT copy + their sem hops from the drain (cross
   term was reduced into out_sb[0,1] earlier; col 1 rows 1: are
   memzero'd).
A post-build legalization pass (_legalize_waits) splits multi-sync-wait
instructions into single-wait NoOp chains because this container's
walrus rejects them.

Key observations used:
 - Only columns 1..8 of attn/attn_erase matter (PROMPT_N=10, token
   channels 1..8); the other 69 columns are dead.
 - The 255/max rescale before the threshold comparison cancels (all
   values nonnegative), and binary {0,255} vs {0,1} is irrelevant
   because only mask>0 is consumed.  Verified margins: min
   |up - thr*max|/max ~ 4e-6 across channels, ~40x the fp32 noise
   between implementations.
 - Bilinear resize is separable: up = U @ img @ U^T with exact
   (binary-fraction) weights; the 256->64 downsample weights are
   exactly {0, 0.5}, so the mask values and the 64->256 matrices are
   exact; mask-side matmuls can run in bf16 exactly.
"""

from contextlib import ExitStack

import numpy as np

H = 64
W = 64
HW = H * W                      # 4096 pixels = mean axis
SEQ = 77
NCORES = 8
COLS = HW // NCORES             # 512 pixel-columns per core
C = 8                           # prompt token channels (seq idx 1..8)
UPS = 256
# streaming chunks of the COLUMN-major (flipped) layout: (jgroup, r0, r1).
# Each chunk is one dma_start loading [128 pixel-columns, x rows r0:r1 |
# y rows r0:r1]; the last chunk is small so the post-DMA drain is tiny.
CHUNKS = [(0, 0, 4096), (1, 0, 4096), (2, 0, 4096), (3, 0, 3968),
          (3, 3968, 4096)]
SUBW = 1024                     # free-dim subtile width for compute overlap
TOKEN_CHANNELS = (1, 4)         # c with seq idx in TOKEN_INDICES=(2,5); c = idx-1
THRS = [0.85 if c in TOKEN_CHANNELS else 0.95 for c in range(C)]
SGNS = [-1.0 if c in TOKEN_CHANNELS else 1.0 for c in range(C)]

# column layout of the merged f32 constant+smalls tensor.  64-partition
# tensors are packed in PAIRS: PE lhsT matrices live at partitions 0:64
# (PE needs base 0 to match its rhs), the cross-attn inputs a8/e8 at
# partitions 64:128 of the same columns (DVE/ACT run fine at base 64).
CM_IDENT = 0            # [0:128)    identity (128,128)
CM_UT = 128             # [128:384)  U^T rows 0:64 (64,256)
CM_ONES = 384           # [384:385)  ones column (128,1)
CM_ONESR = 385          # [385:513)  ones row (1,128)
CM_SGN = 513            # [513:521)  sign row (1,8)
CM_A8T = 521            # [521:1033) partitions 0:64 = a8T (64,512);
                        #            partitions 64:128 = a8 (64,512)
CM_DUP = 1033           # [1033:1161) partitions 0:64 = dup matrix (64,128):
                        #            dup[k, p] = (p==k) + (p==k+64)
CM_UMAXT = 1161         # [1161:1289) partitions 0:64 = U_max^T (64,128): the
                        #            128 extreme rows of U (a in {0,.125,.875,1})
                        #            whose max equals the full 256-row max
CM_UNEEDT = 1289        # [1289:1305) partitions 0:64 = U_need^T (64,16):
                        #            up rows 4R+1, 4R+2 for this core's 8
                        #            pixel rows R (threshold bits live here)
CM_E = 1305             # [1305:1309) partitions 0:16 = E (16,8) bf16 pair-sum
                        #            selector: E[p, r] = (p//2 == r)
                        # [1033:1545) partitions 64:128 = e8 (64,512)
CM_E8 = 1033
CM_W = 1545

_PROG_CACHE = {}
_ABLATE = frozenset()    # bisection: subsets of {'cross','mask','compute'}


def _interp_matrix(out_n, in_n):
    """Row-interpolation matrix of torch bilinear resize (align_corners=False).

    All weights are exact binary fractions for (256,64) and (64,256)."""
    ys = np.clip(
        (np.arange(out_n, dtype=np.float64) + 0.5) * (in_n / out_n) - 0.5,
        0.0, in_n - 1.0,
    )
    y0 = np.floor(ys).astype(np.int64)
    y1 = np.minimum(y0 + 1, in_n - 1)
    wy = ys - y0
    m = np.zeros((out_n, in_n), dtype=np.float32)
    np.add.at(m, (np.arange(out_n), y0), (1.0 - wy).astype(np.float32))
    np.add.at(m, (np.arange(out_n), y1), wy.astype(np.float32))
    return m


def _legalize_waits(nc):
    """Split multi-wait instructions into single-wait NoOp prefixes.

    The walrus build in this container rejects instructions whose ISA
    struct cannot hold all the sync waits Tile assigned (fp32 matmul
    LDWEIGHTS holds one, several v2-lowered structs hold one, the
    kernel-tail Drain holds few).  Engine queues execute in order, so
    hoisting extra waits onto same-engine NoOps (the same pattern the
    all-engine-barrier drains use) preserves semantics.  Matmults
    additionally must not carry DMA-queue-sem waits at all."""
    import concourse.mybir as mybir
    import re

    _MONO_SEM = re.compile(r"^(Pool|Activation|PE|DVE|SP|DMAHW\d|DMASW\d)_\d+$")
    n = 0
    seen = {}  # (engine, sem id) -> max value already waited on that engine
    for f in nc.m.functions:
        for b in f.blocks:
            insts = b.instructions
            out = []
            for inst in insts:
                si = inst.sync_info
                waits = list(si.on_wait) if si and si.on_wait else []
                # drop waits dominated by an earlier same-engine wait
                # (engines execute their queue in order, so sem >= v' with
                # v' >= v implies sem >= v)
                if waits:
                    eng = inst.engine
                    kept = []
                    changed = False
                    for w in waits:
                        kk = (eng, w.id)
                        monotone = bool(_MONO_SEM.match(w.ant_name or ""))
                        if (monotone
                                and getattr(w, "wait_mode", "") == "sem-ge-imm"
                                and w.wait_value is not None
                                and seen.get(kk, -1) >= w.wait_value):
                            changed = True
                            continue
                        kept.append(w)
                        if (monotone
                                and getattr(w, "wait_mode", "") == "sem-ge-imm"
                                and w.wait_value is not None):
                            seen[kk] = max(seen.get(kk, -1), w.wait_value)
                    if changed:
                        inst.sync_info = mybir.SyncInfo(
                            on_wait=kept, on_update=list(si.on_update or []))
                        si = inst.sync_info
                    waits = kept
                is_mm = type(inst).__name__ == "InstMatmult"
                is_isa = type(inst).__name__ == "InstISA"
                mm_dma = is_mm and any(
                    "DMA" in (w.ant_name or "") for w in waits)
                keep, move = waits, []
                if is_isa and waits:
                    # pre-encoded fixed-length blobs cannot carry waits
                    keep, move = [], waits
                elif len(waits) > 1 or mm_dma:
                    eng_w = [w for w in waits if "DMA" not in (w.ant_name or "")]
                    if eng_w:
                        keep = [eng_w[-1]]
                        move = [w for w in waits if w is not keep[0]]
                    else:
                        keep = []
                        move = waits
                if move:
                    for w in move:
                        nop = mybir.InstNoOp(
                            name=f"{inst.name}-lw{n}", ins=[], outs=[],
                            engine=inst.engine)
                        nop.sync_info = mybir.SyncInfo(on_wait=[w], on_update=[])
                        nc.register_instruction(nop)
                        out.append(nop)
                        n += 1
                    inst.sync_info = mybir.SyncInfo(
                        on_wait=keep, on_update=list(si.on_update or []))
                out.append(inst)
            insts[:] = out
    return nc


def _build_program_legalized(repeat=1):
    return _legalize_waits(_build_program_raw(repeat))


def _build_program_raw(repeat=1):
    import concourse.bass as bass
    import concourse.mybir as mybir
    import concourse.tile as tile

    f32 = mybir.dt.float32
    f32r = mybir.dt.float32r
    bf16 = mybir.dt.bfloat16
    OP = mybir.AluOpType
    AF = mybir.ActivationFunctionType

    nc = bass.Bass()

    xy = nc.dram_tensor("xy", [2 * COLS, HW], f32, kind="ExternalInput")
    cm = nc.dram_tensor("cm", [128, CM_W], f32, kind="ExternalInput")
    out = nc.dram_tensor("out", [128, 2 * repeat], f32, kind="ExternalOutput")

    with tile.TileContext(nc) as tc, ExitStack() as ctx:
        consts = ctx.enter_context(tc.tile_pool(name="consts", bufs=1))
        chp = ctx.enter_context(tc.tile_pool(name="chp", bufs=3))
        small = ctx.enter_context(tc.tile_pool(name="small", bufs=2))
        acc = ctx.enter_context(tc.tile_pool(name="acc", bufs=1))
        ps_up = ctx.enter_context(tc.tile_pool(name="ps_up", bufs=3, space="PSUM"))
        ps_sm = ctx.enter_context(tc.tile_pool(name="ps_sm", bufs=2, space="PSUM"))
        ps_pk = ctx.enter_context(tc.tile_pool(name="ps_pk", bufs=3, space="PSUM"))

        cm_sb = consts.tile([128, CM_W], f32)
        nc.gpsimd.dma_start(out=cm_sb, in_=cm[:, :])

        identf_sb = cm_sb[:, CM_IDENT:CM_IDENT + 128]
        ut_sb = cm_sb[0:H, CM_UT:CM_UT + UPS]
        ones_sb = cm_sb[:, CM_ONES:CM_ONES + 1]
        onesr_sb = cm_sb[0:1, CM_ONESR:CM_ONESR + 128]
        sgn_sb = cm_sb[0:1, CM_SGN:CM_SGN + C]
        umaxt_sb = cm_sb[0:H, CM_UMAXT:CM_UMAXT + 128]
        uneedt_sb = cm_sb[0:H, CM_UNEEDT:CM_UNEEDT + 16]
        e_sb = cm_sb[0:16, CM_E:CM_E + 4].bitcast(bf16)
        a8v = cm_sb[H:2 * H, CM_A8T:CM_A8T + W * C]
        e8v = cm_sb[H:2 * H, CM_E8:CM_E8 + W * C]
        a8tt = cm_sb[0:H, CM_A8T:CM_A8T + H * C]
        dup_sb = cm_sb[0:H, CM_DUP:CM_DUP + 128]

        for rep in range(repeat):
            out_sb = acc.tile([128, 2], f32, tag="outsb")
            nc.scalar.memzero(out_sb[:, 1:2])

            # ---- cross-attn term (identical on every core) ----
            do_cross = "cross" not in _ABLATE
            if not do_cross:
                nc.scalar.copy(out_sb[0:1, 1:2], ones_sb[0:1, :])
            if do_cross:
                d8t = small.tile([2 * H, W * C], f32, tag="d8")
                d8 = d8t[H:2 * H, :]
                nc.vector.tensor_tensor(d8, a8v, e8v, OP.subtract)
                s8t = small.tile([2 * H, W * C], f32, tag="s8")
                s8 = s8t[H:2 * H, :]
                nc.scalar.activation(s8, d8, AF.Square)
                cross_ps = ps_sm.tile([1, W * C], f32, tag="ps")
                nc.tensor.matmul(
                    cross_ps, lhsT=ones_sb[H:2 * H, :], rhs=s8,
                    start=True, stop=True)
                cross8 = small.tile([1, C], f32, tag="cross8")
                nc.vector.reduce_sum(
                    out=cross8,
                    in_=cross_ps.rearrange("p (j c) -> p c j", c=C),
                    axis=mybir.AxisListType.X,
                )
                crossw = small.tile([1, C], f32, tag="crossw")
                nc.vector.tensor_tensor(crossw, cross8, sgn_sb, OP.mult)
                nc.vector.reduce_sum(
                    out=out_sb[0:1, 1:2], in_=crossw,
                    axis=mybir.AxisListType.X)

            if "mask" not in _ABLATE:
                # ---- mask pipeline: per-channel upsample + threshold ----
                # up = U @ r has 256 rows, but (a) its global max equals the
                # max over the 128 extreme rows U_max (every interior row is
                # a convex combination in the blend weight of its pair's
                # extreme rows), and (b) the gate = OR over the 2x2 b01
                # block at rows 4R+{1,2} x cols 4w+{1,2} (the 64->256->64
                # resize weights are exactly {0,.5} on those positions), so
                # threshold bits are needed ONLY at this core's 16 rows x
                # 128 cols.  E pair-sums adjacent rows; s2 pair-sums
                # adjacent cols; gate = s2 > 0.
                w8 = acc.tile([C, W], f32, tag="w8")
                a8t_ci = a8tt.rearrange("j (i c) -> j c i", c=C)
                for c in range(C):
                    pk = ps_pk.tile([128, 512], f32, tag="pk")
                    r_ps = pk[0:H, 64:320]
                    nc.tensor.matmul(
                        r_ps, lhsT=a8t_ci[:, c, :], rhs=ut_sb,
                        start=True, stop=True)
                    r_sb = small.tile([H, UPS], f32, tag="r")
                    nc.scalar.copy(r_sb, r_ps)
                    upm_ps = ps_up.tile([128, UPS], f32, tag="up")
                    nc.tensor.matmul(
                        upm_ps, lhsT=umaxt_sb, rhs=r_sb, start=True, stop=True)
                    upn_ps = pk[0:16, 320:448]
                    r_need = r_sb.rearrange(
                        "p (w k) -> p w k", k=4)[:, :, 1:3]
                    nc.tensor.matmul(
                        upn_ps, lhsT=uneedt_sb, rhs=r_need,
                        start=True, stop=True)
                    mxc = small.tile([128, 1], f32, tag="mxc")
                    nc.vector.reduce_max(
                        out=mxc, in_=upm_ps, axis=mybir.AxisListType.X)
                    mxr_ps = ps_sm.tile([1, 128], f32, tag="ps")
                    nc.tensor.transpose(mxr_ps, mxc, identf_sb)
                    mxs = small.tile([1, 1], f32, tag="mxs")
                    nc.vector.reduce_max(
                        out=mxs, in_=mxr_ps, axis=mybir.AxisListType.X)
                    ts_sb = small.tile([1, 1], f32, tag="ts")
                    nc.vector.tensor_scalar_mul(
                        ts_sb, in0=mxs, scalar1=float(THRS[c]))
                    tb_ps = ps_sm.tile([16, 1], f32, tag="ps")
                    nc.tensor.matmul(
                        tb_ps, lhsT=onesr_sb[:, 0:16], rhs=ts_sb,
                        start=True, stop=True)
                    tthr = small.tile([16, 1], f32, tag="tthr")
                    nc.scalar.copy(tthr, tb_ps)
                    b01c = small.tile([16, 128], bf16, tag="b01c")
                    nc.vector.tensor_scalar(
                        out=b01c, in0=upn_ps,
                        scalar1=tthr, scalar2=None, op0=OP.is_ge)
                    m8_ps = ps_sm.tile([C, 128], f32, tag="ps")
                    nc.tensor.matmul(
                        m8_ps, lhsT=e_sb, rhs=b01c, start=True, stop=True)
                    m8_sb = small.tile([C, 128], f32, tag="m8")
                    nc.scalar.copy(m8_sb, m8_ps)
                    m8v = m8_sb.rearrange("r (w k) -> r w k", k=2)
                    s2 = small.tile([C, W], f32, tag="s2")
                    nc.vector.tensor_tensor(
                        s2, m8v[:, :, 0], m8v[:, :, 1], OP.add)
                    g2 = small.tile([C, W], f32, tag="g2")
                    nc.vector.tensor_scalar(
                        out=g2, in0=s2, scalar1=0.0, scalar2=None,
                        op0=OP.is_gt)
                    if c == 0:
                        nc.gpsimd.tensor_copy(w8, g2)
                    else:
                        nc.gpsimd.tensor_tensor(
                            w8, w8, g2,
                            OP.add if SGNS[c] > 0 else OP.subtract)

                # flip w8 [8,64] into wfl [128,4] (partition p = pixel
                # column j%128, col g = j//128; j = r*64+w ->
                # wfl[p,g] = w8[2g + p//64, p%64]).  PE transpose -> SBUF,
                # dup-matmul broadcasts both partition halves, then two
                # strided DVE copies pick even/odd source rows.
                w8t_ps = ps_sm.tile([H, C], f32, tag="ps")
                nc.tensor.transpose(w8t_ps, w8, identf_sb[0:C, 0:C])
                w8t_sb = small.tile([H, C], f32, tag="w8t")
                nc.vector.tensor_copy(w8t_sb, w8t_ps)
                w8b_ps = ps_sm.tile([128, C], f32, tag="ps")
                nc.tensor.matmul(
                    w8b_ps, lhsT=dup_sb, rhs=w8t_sb, start=True, stop=True)
                wfl = acc.tile([128, 4], f32, tag="wfl")
                w8b_v = w8b_ps.rearrange("p (g k) -> p k g", k=2)
                nc.vector.tensor_copy(wfl[0:64, :], w8b_v[0:64, 0, :])
                nc.vector.tensor_copy(wfl[64:128, :], w8b_v[64:128, 1, :])

            # ---- main streaming loop: column sums of (x-y)^2 ----
            # Flipped layout: partition p = pixel column j = g*128+p, free
            # dim = hw rows.  One dma_start per chunk loads [x rows | y
            # rows] (2 contiguous descriptors per partition).  Subtract in
            # place (DVE/Pool alternating), then ACT Square with accum_out
            # reduces along the free dim -- no PE matmuls in the bulk path
            # at all (PE idles at the 1.2 GHz pstate, so PE work is
            # precious; the mask pipeline owns it).
            sg = acc.tile([128, 4], f32, tag="sg")
            xyv = xy.rearrange("(t p) r -> t p r", t=2)
            do_comp = "compute" not in _ABLATE
            for ci, (g, r0, r1) in enumerate(CHUNKS):
                R = r1 - r0
                cht = chp.tile([128, 2, R], f32, tag="ch")
                nc.sync.dma_start(
                    out=cht,
                    in_=xyv[:, g * 128:(g + 1) * 128, r0:r1].rearrange(
                        "t p r -> p t r"))
                if not do_comp:
                    continue
                xt = cht[:, 0]
                yt = cht[:, 1]
                o = 0
                si = 0
                while o < R:
                    wdt = min(SUBW, R - o)
                    if R - o - wdt < SUBW // 2 and R - o - wdt > 0:
                        wdt = R - o  # avoid tiny tail subtiles
                    sub_eng = nc.vector if (ci + si) % 2 == 0 else nc.gpsimd
                    sub_eng.tensor_tensor(
                        xt[:, o:o + wdt], xt[:, o:o + wdt], yt[:, o:o + wdt],
                        OP.subtract)
                    pt = small.tile([128, 1], f32, tag="pt")
                    nc.scalar.activation(
                        xt[:, o:o + wdt], xt[:, o:o + wdt], AF.Square,
                        accum_out=pt)
                    if ci == len(CHUNKS) - 1:
                        pt_last = pt  # folded in the drain, not into sg
                    elif r0 == 0 and o == 0:
                        nc.gpsimd.tensor_copy(sg[:, g:g + 1], pt)
                    else:
                        nc.gpsimd.tensor_tensor(
                            sg[:, g:g + 1], sg[:, g:g + 1], pt, OP.add)
                    o += wdt
                    si += 1

            # ---- dot with the flipped gate grid; short drain tail ----
            # per-partition partials go straight to DRAM; the host sums the
            # 128 values -- no PE partition-sum or ACT copy on the drain.
            # The dot over sg (all chunks but the last) is precomputed as
            # basep while the last chunk streams, so the drain is just two
            # in-order DVE ops folding pt_last.
            if not do_comp:
                nc.vector.tensor_copy(out_sb[:, 0:1], ones_sb)
            elif "mask" in _ABLATE:
                tmpm = acc.tile([128, 1], f32, tag="tmpm")
                nc.vector.reduce_sum(
                    out=tmpm, in_=sg, axis=mybir.AxisListType.X)
                nc.vector.tensor_tensor(
                    out_sb[:, 0:1], tmpm, pt_last, OP.add)
            else:
                sw = acc.tile([128, 4], f32, tag="sw")
                basep = acc.tile([128, 1], f32, tag="basep")
                nc.vector.tensor_tensor(sw, sg, wfl, OP.mult)
                nc.vector.reduce_sum(
                    out=basep, in_=sw, axis=mybir.AxisListType.X)
                tmp3 = acc.tile([128, 1], f32, tag="tmp3")
                nc.vector.tensor_tensor(tmp3, pt_last, wfl[:, 3:4], OP.mult)
                nc.vector.tensor_tensor(
                    out_sb[:, 0:1], tmp3, basep, OP.add)
            nc.sync.dma_start(out=out[:, 2 * rep:2 * rep + 2], in_=out_sb)

    return nc


def _build_program(repeat=1):
    return _build_program_legalized(repeat)


def _get_program(repeat=1):
    key = ("nc", repeat)
    if key not in _PROG_CACHE:
        _PROG_CACHE[key] = _build_program(repeat)
    return _PROG_CACHE[key]


def _host_constants():
    """Static part of the merged cm tensor, one per core (uneedt differs)."""
    if "cms" in _PROG_CACHE:
        return _PROG_CACHE["cms"]
    from concourse import mybir

    np_bf16 = mybir.dt.np(mybir.dt.bfloat16)
    u = _interp_matrix(UPS, H)          # (256, 64) upsample

    base = np.zeros((128, CM_W), dtype=np.float32)
    base[:, CM_IDENT:CM_IDENT + 128] = np.eye(128, dtype=np.float32)
    base[0:H, CM_UT:CM_UT + UPS] = u.T
    base[:, CM_ONES] = 1.0
    base[0, CM_ONESR:CM_ONESR + 128] = 1.0
    base[0, CM_SGN:CM_SGN + C] = np.asarray(SGNS, dtype=np.float32)
    dup = np.zeros((H, 128), dtype=np.float32)
    dup[np.arange(H), np.arange(H)] = 1.0
    dup[np.arange(H), np.arange(H) + H] = 1.0
    base[0:H, CM_DUP:CM_DUP + 128] = dup
    # the 128 extreme rows of U: pure rows 0/255 plus both a in {.125,.875}
    # rows of every adjacent pair -- their max equals the full 256-row max
    uidx = [0, 255] + [4 * k + 2 for k in range(63)] + [4 * k + 5
                                                        for k in range(63)]
    base[0:H, CM_UMAXT:CM_UMAXT + 128] = u[uidx].T
    ev = np.zeros((16, 8), dtype=np.float32)
    ev[np.arange(16), np.arange(16) // 2] = 1.0
    base[0:16, CM_E:CM_E + 4] = np.ascontiguousarray(
        ev.astype(np_bf16)).view(np.float32)

    cms = []
    for core in range(NCORES):
        cm = base.copy()
        need = [u[4 * (C * core + rr) + 1 + t] for rr in range(C)
                for t in range(2)]
        cm[0:H, CM_UNEEDT:CM_UNEEDT + 16] = np.stack(need).T
        cms.append(cm)
    _PROG_CACHE["cms"] = cms
    return cms


def _make_in_maps(inputs):
    attn = np.ascontiguousarray(inputs["attn"], dtype=np.float32)
    attn_erase = np.ascontiguousarray(inputs["attn_erase"], dtype=np.float32)
    sa = np.asarray(inputs["self_attn"], dtype=np.float32).reshape(HW, HW)
    sae = np.asarray(
        inputs["self_attn_erase"], dtype=np.float32).reshape(HW, HW)

    a8 = np.ascontiguousarray(attn[:, :, 1:1 + C]).reshape(H, W * C)
    a8t_host = np.ascontiguousarray(
        attn[:, :, 1:1 + C].transpose(1, 0, 2)).reshape(W, H * C)
    e8 = np.ascontiguousarray(attn_erase[:, :, 1:1 + C]).reshape(H, W * C)
    cms = _host_constants()

    saT = np.ascontiguousarray(sa.T)    # [hw columns, hw rows]
    saeT = np.ascontiguousarray(sae.T)
    in_maps = []
    for core in range(NCORES):
        cm = cms[core].copy()
        cm[0:H, CM_A8T:CM_A8T + H * C] = a8t_host
        cm[H:2 * H, CM_A8T:CM_A8T + W * C] = a8
        cm[H:2 * H, CM_E8:CM_E8 + W * C] = e8
        xyc = np.empty((2 * COLS, HW), dtype=np.float32)
        xyc[0:COLS] = saT[core * COLS:(core + 1) * COLS]
        xyc[COLS:] = saeT[core * COLS:(core + 1) * COLS]
        in_maps.append({"xy": xyc, "cm": cm})
    return in_maps


def _combine(outs):
    self_raw = sum(float(np.asarray(o[:, 0], dtype=np.float64).sum())
                   for o in outs)
    cross_raw = float(outs[0][0, 1])
    return np.float32((self_raw + cross_raw) / float(HW))


def kernel(**inputs):
    from concourse.bass_utils import run_bass_kernel_spmd

    nc = _get_program()
    in_maps = _make_in_maps(inputs)
    res = run_bass_kernel_spmd(nc, in_maps, core_ids=list(range(NCORES)))
    return _combine([r["out"] for r in res.results])

